# revision 3
# baseline (speedup 1.0000x reference)
"""CrossAttentionFusion Trainium2 kernel.

Data-parallel over batch: 64 graphs -> 8 NeuronCores x 8 graphs.
Each core holds the full weights and runs dense padded cross-attention
(mol queries 128, target keys/values 1024, D=512, H=4 heads of 128),
LayerNorm + mean-pool + fusion FFN.

Layout conventions on device (per graph g, head h):
  tgtT   [din,  j]   target activations transposed (4 chunks of 128 din rows)
  molT   [din,  i]   mol activations transposed, 8 graphs side by side
  QT     [dout, i]   = Wq.T-chunks.T @ molT   (bias bq added on PSUM->SBUF copy)
  KT     [dout, j]   per head [128, 1024]     (bias bk added on copy)
  V      [j, dout]   natural layout           (bias bv folded via softmax-sum=1
                                               into the output-projection bias)
  scoresT[j, i]      = KT_h-chunk.T @ QT_h    (k = hd)
  expT   [j, i]      = Exp(scoresT / sqrt(hd))  (no max-subtraction; scores are
                                               O(1) so exp is safe in fp32)
  d      [1, i]      = ones_col.T @ expT      softmax denominator per (h)
  attT_u [hd, i]     = V_h-chunk.T @ expT     unnormalized attention output
  r_bcast[*, (h,i)]  rank-1 matmul 0.25*(1/d) broadcast down partitions
  attT_n             = attT_u * r_bcast       (x4 compensation on Wo copy)
  att2   [i, dout]   = attT_n-chunks.T @ Wo.T-chunks
  x      [i, dout]   = 4*att2 + (mol_pad + bo + bv@Wo.T)   -> LayerNorm -> h_tilde
  w_meanT[j, i]      = sum_h expT_h * r_bcast_h  (0.25 folded into r_bcast)
  pooledT[d, 1]      = h_tilde-chunk.T @ ones/128
  FFN: h1T [dout,g] = Wf1.T-chunks.T @ fusedT  (+bf1 per-partition, exact Gelu)
       out  [g,dout] = h1gT-chunks.T @ Wf2.T-chunks (+bf2)

kernel(**inputs) takes the FULL unsharded inputs and returns
(fused [64,512], h_tilde [64,128,512], w_mean [64,128,1024]) like the reference.
"""

import sys

if "/opt/trn_rl_repo" not in sys.path:
    sys.path.insert(0, "/opt/trn_rl_repo")

import numpy as np

import concourse.bass as bass
import concourse.mybir as mybir
import concourse.tile as tile
from concourse import bacc
from concourse.bass_utils import run_bass_kernel_spmd

F32 = mybir.dt.float32
AF = mybir.ActivationFunctionType
ALU = mybir.AluOpType

B = 64
D = 512
H = 4
HD = 128
NM = 128          # max_mol
NT = 1024         # max_target
NCORES = 8
GPC = B // NCORES  # graphs per core
KC = D // 128      # contraction chunks of 128 over D
LN_EPS = 1e-5
EXP_SCALE = 1.0 / np.sqrt(np.float32(HD))

_CACHE: dict = {}


def _bcast(ap, p):
    """Broadcast a 1-D DRAM AP down p partitions (stride-0 partition dim)."""
    return bass.AP(tensor=ap.tensor, offset=ap.offset, ap=[[0, p]] + list(ap.ap))



def _build_nc():
    nc = bacc.Bacc("TRN2", target_bir_lowering=False, debug=True)

    # ---- DRAM I/O (per core) ----
    d_tgtT = nc.dram_tensor("tgtT", [GPC, KC, 128, NT], F32, kind="ExternalInput")
    d_molT = nc.dram_tensor("molT", [KC, 128, GPC * NM], F32, kind="ExternalInput")
    d_molp = nc.dram_tensor("molp", [GPC, NM, D], F32, kind="ExternalInput")
    d_wqT = nc.dram_tensor("wqT", [KC, 128, D], F32, kind="ExternalInput")
    d_wkT = nc.dram_tensor("wkT", [KC, 128, D], F32, kind="ExternalInput")
    d_wvT = nc.dram_tensor("wvT", [KC, 128, D], F32, kind="ExternalInput")
    d_woT = nc.dram_tensor("woT", [KC, 128, D], F32, kind="ExternalInput")
    d_wf1T = nc.dram_tensor("wf1T", [3 * KC, 128, D], F32, kind="ExternalInput")
    d_wf2T = nc.dram_tensor("wf2T", [KC, 128, D], F32, kind="ExternalInput")
    d_bq = nc.dram_tensor("bq", [KC, 128, 1], F32, kind="ExternalInput")
    d_bk = nc.dram_tensor("bk", [KC, 128, 1], F32, kind="ExternalInput")
    d_bf1 = nc.dram_tensor("bf1", [KC, 128, 1], F32, kind="ExternalInput")
    d_bf2 = nc.dram_tensor("bf2", [D], F32, kind="ExternalInput")
    d_lng = nc.dram_tensor("lng", [D], F32, kind="ExternalInput")
    d_lnb = nc.dram_tensor("lnb", [D], F32, kind="ExternalInput")
    d_geT = nc.dram_tensor("geT", [2 * KC, 128, GPC], F32, kind="ExternalInput")

    d_ht = nc.dram_tensor("h_tilde", [GPC, NM, D], F32, kind="ExternalOutput")
    # (g, half, j_within_chunk, jc_local, i)
    d_wm = nc.dram_tensor("w_meanT", [GPC, 2, 128, 4, 128], F32, kind="ExternalOutput")
    d_fused = nc.dram_tensor("fused", [GPC, D], F32, kind="ExternalOutput")

    with tile.TileContext(nc) as tc:
        with (
            tc.tile_pool(name="wpool", bufs=1) as wp,
            tc.tile_pool(name="mpool", bufs=1) as mp,
            tc.tile_pool(name="gpool", bufs=2) as gp,
            tc.tile_pool(name="pspool", bufs=2, space="PSUM") as pp,
        ):
            # ---- resident weights/constants ----
            wq = [wp.tile([128, D], F32, name=f"wq{c}", tag="wq", bufs=KC) for c in range(KC)]
            wk = [wp.tile([128, D], F32, name=f"wk{c}", tag="wk", bufs=KC) for c in range(KC)]
            wv = [wp.tile([128, D], F32, name=f"wv{c}", tag="wv", bufs=KC) for c in range(KC)]
            wo = [wp.tile([128, D], F32, name=f"wo{c}", tag="wo", bufs=KC) for c in range(KC)]
            wf2 = [wp.tile([128, D], F32, name=f"wf2_{c}", tag="wf2", bufs=KC) for c in range(KC)]
            for c in range(KC):
                nc.sync.dma_start(out=wq[c][:], in_=d_wqT[c])
                nc.sync.dma_start(out=wk[c][:], in_=d_wkT[c])
                nc.sync.dma_start(out=wv[c][:], in_=d_wvT[c])
                nc.sync.dma_start(out=wo[c][:], in_=d_woT[c])
                nc.sync.dma_start(out=wf2[c][:], in_=d_wf2T[c])
            bq = [wp.tile([128, 1], F32, name=f"bq{c}", tag="bq", bufs=KC) for c in range(KC)]
            bk = [wp.tile([128, 1], F32, name=f"bk{c}", tag="bk", bufs=KC) for c in range(KC)]
            bf1 = [wp.tile([128, 1], F32, name=f"bf1_{c}", tag="bf1", bufs=KC) for c in range(KC)]
            for c in range(KC):
                nc.sync.dma_start(out=bq[c][:], in_=d_bq[c])
                nc.sync.dma_start(out=bk[c][:], in_=d_bk[c])
                nc.sync.dma_start(out=bf1[c][:], in_=d_bf1[c])
            lng = wp.tile([128, D], F32, tag="lng")
            lnb = wp.tile([128, D], F32, tag="lnb")
            nc.sync.dma_start(out=lng[:], in_=_bcast(d_lng[:], 128))
            nc.sync.dma_start(out=lnb[:], in_=_bcast(d_lnb[:], 128))
            bf2b = wp.tile([GPC, D], F32, tag="bf2b")
            nc.sync.dma_start(out=bf2b[:], in_=_bcast(d_bf2[:], GPC))
            geT = [wp.tile([128, GPC], F32, name=f"geT{c}", tag="geT", bufs=2 * KC) for c in range(2 * KC)]
            for c in range(2 * KC):
                nc.sync.dma_start(out=geT[c][:], in_=d_geT[c])

            eps_t = wp.tile([128, 1], F32, tag="eps")
            nc.vector.memset(eps_t[:], LN_EPS)
            ones_col = wp.tile([128, 1], F32, tag="ones")
            nc.vector.memset(ones_col[:], 1.0)
            inv128_col = wp.tile([128, 1], F32, tag="inv128")
            nc.vector.memset(inv128_col[:], 1.0 / NM)
            q_row = wp.tile([1, 128], F32, tag="qrow")
            nc.vector.memset(q_row[:], 0.25)

            # fusedT pooled chunks, filled per graph
            fpool = [wp.tile([128, GPC], F32, name=f"fpool{c}", tag="fpool", bufs=KC) for c in range(KC)]

            # ---- molT + QT for all graphs ----
            molT = [mp.tile([128, GPC * NM], F32, name=f"molT{c}", tag="molT", bufs=KC) for c in range(KC)]
            for c in range(KC):
                nc.sync.dma_start(out=molT[c][:], in_=d_molT[c])
            qt = [mp.tile([128, GPC * NM], F32, name=f"qt{c}", tag="qt", bufs=H) for c in range(H)]
            for hc in range(H):
                for half in range(2):
                    ps = pp.tile([128, 512], F32, tag="proj")
                    for c in range(KC):
                        nc.tensor.matmul(
                            ps[:],
                            lhsT=wq[c][:, hc * 128 : (hc + 1) * 128],
                            rhs=molT[c][:, half * 512 : (half + 1) * 512],
                            start=(c == 0),
                            stop=(c == KC - 1),
                        )
                    nc.scalar.activation(
                        out=qt[hc][:, half * 512 : (half + 1) * 512],
                        in_=ps[:],
                        func=AF.Copy,
                        bias=0.0,
                        scale=1.0,
                    )
                    # add bq along partitions (dout) via tensor_scalar
                    nc.vector.tensor_scalar_add(
                        out=qt[hc][:, half * 512 : (half + 1) * 512],
                        in0=qt[hc][:, half * 512 : (half + 1) * 512],
                        scalar1=bq[hc][:],
                    )

            # ---- per-graph pipeline ----
            for g in range(GPC):
                tgt = [gp.tile([128, NT], F32, name=f"tgt{c}", tag="tgt", bufs=5) for c in range(KC)]
                for c in range(KC):
                    nc.sync.dma_start(out=tgt[c][:], in_=d_tgtT[g, c])

                # K^T per head [dout=hd, j]
                kt = [gp.tile([128, NT], F32, name=f"kt{c}", tag="kt", bufs=4) for c in range(H)]
                for h in range(H):
                    for half in range(2):
                        ps = pp.tile([128, 512], F32, tag="proj")
                        for c in range(KC):
                            nc.tensor.matmul(
                                ps[:],
                                lhsT=wk[c][:, h * 128 : (h + 1) * 128],
                                rhs=tgt[c][:, half * 512 : (half + 1) * 512],
                                start=(c == 0),
                                stop=(c == KC - 1),
                            )
                        # copy + bias bk (per-partition dout)
                        nc.scalar.activation(
                            out=kt[h][:, half * 512 : (half + 1) * 512],
                            in_=ps[:],
                            func=AF.Identity,
                            bias=bk[h][:],
                            scale=1.0,
                        )

                # V natural [j, dout], no bias (folded downstream)
                vv = [gp.tile([128, D], F32, name=f"vv{c}", tag="vv", bufs=10) for c in range(8)]
                for jc in range(8):
                    ps = pp.tile([128, 512], F32, tag="proj")
                    for c in range(KC):
                        nc.tensor.matmul(
                            ps[:],
                            lhsT=tgt[c][:, jc * 128 : (jc + 1) * 128],
                            rhs=wv[c][:],
                            start=(c == 0),
                            stop=(c == KC - 1),
                        )
                    nc.vector.tensor_copy(out=vv[jc][:], in_=ps[:])

                # scoresT + exp: per head, 8 j-chunks -> 2 sbuf tiles of [128, 4*128]
                ex = [gp.tile([128, 512], F32, name=f"ex{c}", tag="ex", bufs=10) for c in range(8)]
                # ex[h*2 + q] covers j-chunks q*4..q*4+3 for head h
                for h in range(H):
                    for q in range(2):
                        ps = pp.tile([128, 512], F32, tag="sc")
                        for jl in range(4):
                            jc = q * 4 + jl
                            nc.tensor.matmul(
                                ps[:, jl * 128 : (jl + 1) * 128],
                                lhsT=kt[h][:, jc * 128 : (jc + 1) * 128],
                                rhs=qt[h][:, g * 128 : (g + 1) * 128],
                                start=True,
                                stop=True,
                            )
                        nc.scalar.activation(
                            out=ex[h * 2 + q][:],
                            in_=ps[:],
                            func=AF.Exp,
                            bias=0.0,
                            scale=float(EXP_SCALE),
                        )

                # attT_u (accum over j) + softmax denominators
                att_ps = pp.tile([128, 512], F32, tag="att")
                sm_ps = pp.tile([128, 512], F32, tag="sm", bufs=1)
                for h in range(H):
                    for jc in range(8):
                        e_sl = ex[h * 2 + jc // 4][:, (jc % 4) * 128 : (jc % 4 + 1) * 128]
                        nc.tensor.matmul(
                            att_ps[:, h * 128 : (h + 1) * 128],
                            lhsT=vv[jc][:, h * 128 : (h + 1) * 128],
                            rhs=e_sl,
                            start=(jc == 0),
                            stop=(jc == 7),
                        )
                        nc.tensor.matmul(
                            sm_ps[0:1, h * 128 : (h + 1) * 128],
                            lhsT=ones_col[:],
                            rhs=e_sl,
                            start=(jc == 0),
                            stop=(jc == 7),
                        )

                # r = 0.25/d broadcast down partitions (rank-1 matmul)
                rrow = gp.tile([1, 512], F32, tag="rrow")
                nc.vector.reciprocal(out=rrow[:], in_=sm_ps[0:1, :])
                nc.tensor.matmul(
                    sm_ps[:], lhsT=q_row[:], rhs=rrow[:], start=True, stop=True
                )
                rbc = gp.tile([128, 512], F32, tag="rbc")
                nc.scalar.activation(
                    out=rbc[:], in_=sm_ps[:], func=AF.Copy, bias=0.0, scale=1.0
                )

                # normalize attention (0.25*r; compensated by 4x later)
                attn = gp.tile([128, 512], F32, tag="attn")
                nc.vector.tensor_tensor(
                    out=attn[:], in0=att_ps[:], in1=rbc[:], op=ALU.mult
                )

                # output projection + residual + LN
                att2_ps = pp.tile([128, 512], F32, tag="att")
                for c in range(KC):
                    nc.tensor.matmul(
                        att2_ps[:],
                        lhsT=attn[:, c * 128 : (c + 1) * 128],
                        rhs=wo[c][:],
                        start=(c == 0),
                        stop=(c == KC - 1),
                    )
                molp = gp.tile([128, D], F32, tag="molp")
                nc.sync.dma_start(out=molp[:], in_=d_molp[g])
                x = gp.tile([128, D], F32, tag="x")
                nc.scalar.activation(
                    out=x[:], in_=att2_ps[:], func=AF.Copy, bias=0.0, scale=4.0
                )
                nc.vector.tensor_tensor(out=x[:], in0=x[:], in1=molp[:], op=ALU.add)

                stats = gp.tile([128, 6], F32, tag="stats")
                nc.vector.bn_stats(out=stats[:], in_=x[:])
                mv = gp.tile([128, 2], F32, tag="mv")
                nc.vector.bn_aggr(out=mv[:], in_=stats[:])
                std = gp.tile([128, 1], F32, tag="std")
                nc.scalar.activation(
                    out=std[:], in_=mv[:, 1:2], func=AF.Sqrt, bias=eps_t[:], scale=1.0
                )
                nc.vector.reciprocal(out=std[:], in_=std[:])
                h_t = gp.tile([128, D], F32, tag="ht")
                nc.vector.tensor_scalar(
                    out=h_t[:],
                    in0=x[:],
                    scalar1=mv[:, 0:1],
                    scalar2=std[:],
                    op0=ALU.subtract,
                    op1=ALU.mult,
                )
                nc.gpsimd.tensor_tensor(out=h_t[:], in0=h_t[:], in1=lng[:], op=ALU.mult)
                nc.gpsimd.tensor_tensor(out=h_t[:], in0=h_t[:], in1=lnb[:], op=ALU.add)
                nc.sync.dma_start(out=d_ht[g], in_=h_t[:])

                # pooledT column g of fusedT chunks 8..11
                pool_ps = pp.tile([128, KC], F32, tag="pool", bufs=1)
                for c in range(KC):
                    nc.tensor.matmul(
                        pool_ps[:, c : c + 1],
                        lhsT=h_t[:, c * 128 : (c + 1) * 128],
                        rhs=inv128_col[:],
                        start=True,
                        stop=True,
                    )
                for c in range(KC):
                    nc.scalar.activation(
                        out=fpool[c][:, g : g + 1],
                        in_=pool_ps[:, c : c + 1],
                        func=AF.Copy,
                        bias=0.0,
                        scale=1.0,
                    )

                # w_meanT = sum_h expT_h * rbc_h  (0.25 already folded)
                for q in range(2):
                    wm = gp.tile([128, 512], F32, tag="wm", bufs=4)
                    for jl in range(4):
                        sl = slice(jl * 128, (jl + 1) * 128)
                        t0 = gp.tile([128, 128], F32, tag="wmt", bufs=4)
                        nc.vector.tensor_tensor(
                            out=wm[:, sl], in0=ex[0 * 2 + q][:, sl],
                            in1=rbc[:, 0:128], op=ALU.mult,
                        )
                        nc.vector.tensor_tensor(
                            out=t0[:], in0=ex[1 * 2 + q][:, sl],
                            in1=rbc[:, 128:256], op=ALU.mult,
                        )
                        nc.gpsimd.tensor_tensor(
                            out=wm[:, sl], in0=wm[:, sl], in1=t0[:], op=ALU.add
                        )
                        t1 = gp.tile([128, 128], F32, tag="wmt", bufs=4)
                        nc.vector.tensor_tensor(
                            out=t1[:], in0=ex[2 * 2 + q][:, sl],
                            in1=rbc[:, 256:384], op=ALU.mult,
                        )
                        nc.gpsimd.tensor_tensor(
                            out=wm[:, sl], in0=wm[:, sl], in1=t1[:], op=ALU.add
                        )
                        t2 = gp.tile([128, 128], F32, tag="wmt", bufs=4)
                        nc.vector.tensor_tensor(
                            out=t2[:], in0=ex[3 * 2 + q][:, sl],
                            in1=rbc[:, 384:512], op=ALU.mult,
                        )
                        nc.gpsimd.tensor_tensor(
                            out=wm[:, sl], in0=wm[:, sl], in1=t2[:], op=ALU.add
                        )
                    nc.sync.dma_start(
                        out=d_wm[g, q], in_=wm[:].rearrange("p (a b) -> p a b", a=4)
                    )

            # ---- FFN over the 8 pooled rows ----
            wf1 = [gp.tile([128, D], F32, name=f"wf1a{c}", tag="ex", bufs=10) for c in range(6)] + [
                gp.tile([128, D], F32, name=f"wf1b{c}", tag="vv", bufs=10) for c in range(6)
            ]
            for c in range(3 * KC):
                nc.sync.dma_start(out=wf1[c][:], in_=d_wf1T[c])
            fT = geT + fpool  # 12 chunks of [128, GPC]
            h1g = []
            for m in range(KC):
                ps = pp.tile([128, GPC], F32, tag="proj")
                for c in range(3 * KC):
                    nc.tensor.matmul(
                        ps[:],
                        lhsT=wf1[c][:, m * 128 : (m + 1) * 128],
                        rhs=fT[c][:],
                        start=(c == 0),
                        stop=(c == 3 * KC - 1),
                    )
                hg = gp.tile([128, GPC], F32, tag="h1g", bufs=4)
                nc.scalar.activation(
                    out=hg[:], in_=ps[:], func=AF.Gelu, bias=bf1[m][:], scale=1.0
                )
                h1g.append(hg)
            out2_ps = pp.tile([GPC, 512], F32, tag="att")
            for c in range(KC):
                nc.tensor.matmul(
                    out2_ps[:],
                    lhsT=h1g[c][:],
                    rhs=wf2[c][:],
                    start=(c == 0),
                    stop=(c == KC - 1),
                )
            fused_sb = gp.tile([GPC, D], F32, tag="fused")
            nc.vector.tensor_tensor(
                out=fused_sb[:], in0=out2_ps[:], in1=bf2b[:], op=ALU.add
            )
            nc.sync.dma_start(out=d_fused[:], in_=fused_sb[:])

    nc.compile()
    return nc


def _prep_host(inputs):
    mol_node = np.asarray(inputs["mol_node_emb"], np.float32)
    tgt_node = np.asarray(inputs["target_node_emb"], np.float32)
    mol_ge = np.asarray(inputs["mol_graph_emb"], np.float32)
    tgt_ge = np.asarray(inputs["target_graph_emb"], np.float32)
    Wq = np.asarray(inputs["Wq"], np.float32)
    bq = np.asarray(inputs["bq"], np.float32)
    Wk = np.asarray(inputs["Wk"], np.float32)
    bk = np.asarray(inputs["bk"], np.float32)
    Wv = np.asarray(inputs["Wv"], np.float32)
    bv = np.asarray(inputs["bv"], np.float32)
    Wo = np.asarray(inputs["Wo"], np.float32)
    bo = np.asarray(inputs["bo"], np.float32)
    ln_g = np.asarray(inputs["ln_g"], np.float32)
    ln_b = np.asarray(inputs["ln_b"], np.float32)
    Wf1 = np.asarray(inputs["Wf1"], np.float32)
    bf1 = np.asarray(inputs["bf1"], np.float32)
    Wf2 = np.asarray(inputs["Wf2"], np.float32)
    bf2 = np.asarray(inputs["bf2"], np.float32)
    mol_batch = np.asarray(inputs["mol_batch"])
    mol_pos = np.asarray(inputs["mol_pos"])
    tgt_batch = np.asarray(inputs["target_batch"])
    tgt_pos = np.asarray(inputs["target_pos"])

    mol_pad = np.zeros((B, NM, D), np.float32)
    mol_pad[mol_batch, mol_pos] = mol_node
    tgt_pad = np.zeros((B, NT, D), np.float32)
    tgt_pad[tgt_batch, tgt_pos] = tgt_node

    bo_eff = bo + bv @ Wo.T                      # softmax rows sum to 1
    molp = (mol_pad + bo_eff).astype(np.float32)  # residual + folded biases

    wqT = np.ascontiguousarray(Wq.T).reshape(KC, 128, D)
    wkT = np.ascontiguousarray(Wk.T).reshape(KC, 128, D)
    wvT = np.ascontiguousarray(Wv.T).reshape(KC, 128, D)
    woT = np.ascontiguousarray(Wo.T).reshape(KC, 128, D)
    wf1T = np.ascontiguousarray(Wf1.T).reshape(3 * KC, 128, D)
    wf2T = np.ascontiguousarray(Wf2.T).reshape(KC, 128, D)

    in_maps = []
    for c in range(NCORES):
        gs = slice(c * GPC, (c + 1) * GPC)
        mT = np.ascontiguousarray(
            mol_pad[gs].transpose(2, 0, 1).reshape(KC, 128, GPC * NM)
        )
        tT = np.ascontiguousarray(tgt_pad[gs].transpose(0, 2, 1)).reshape(
            GPC, KC, 128, NT
        )
        ge = np.concatenate([mol_ge[gs], tgt_ge[gs]], axis=1)  # [GPC, 2D]
        geT = np.ascontiguousarray(ge.T).reshape(2 * KC, 128, GPC)
        in_maps.append(
            {
                "tgtT": tT,
                "molT": mT,
                "molp": np.ascontiguousarray(molp[gs]),
                "wqT": wqT,
                "wkT": wkT,
                "wvT": wvT,
                "woT": woT,
                "wf1T": wf1T,
                "wf2T": wf2T,
                "bq": bq.reshape(KC, 128, 1),
                "bk": bk.reshape(KC, 128, 1),
                "bf1": bf1.reshape(KC, 128, 1),
                "bf2": bf2,
                "lng": ln_g,
                "lnb": ln_b,
                "geT": geT,
            }
        )
    return in_maps


def kernel(**inputs):
    if "nc" not in _CACHE:
        _CACHE["nc"] = _build_nc()
    nc = _CACHE["nc"]
    in_maps = _prep_host(inputs)
    res = run_bass_kernel_spmd(nc, in_maps, list(range(NCORES))).results

    fused = np.concatenate([res[c]["fused"] for c in range(NCORES)], axis=0)
    h_tilde = np.concatenate([res[c]["h_tilde"] for c in range(NCORES)], axis=0)
    wmT = np.concatenate([res[c]["w_meanT"] for c in range(NCORES)], axis=0)
    # (g, q, j_w, jc_l, i) -> (g, i, q, jc_l, j_w) -> [B, NM, NT]
    w_mean = np.ascontiguousarray(wmT.transpose(0, 4, 1, 3, 2)).reshape(B, NM, NT)
    return fused, h_tilde, w_mean
